# revision 1
# baseline (speedup 1.0000x reference)
"""DLRM inference kernel for 8 Trainium2 NeuronCores.

Strategy: pure data parallelism (batch 16384 -> 8 x 2048; tables + weights
replicated, no collectives). All on-chip compute in bf16 (tolerance 2e-2).

Per-core pipeline:
  1. Embedding gather: ONE batched indirect DMA per 128-sample tile
     (26-column offset AP) from bf16 tables in DRAM.
  2. Feature-major layout via DMA-xbar transposes: 27 OVERLAPPED tiles,
     tile t = features (t, t+1) stacked on 2x64 partitions. The bottom-MLP
     output (computed feature-major natively) is copied into tile 0's lower
     half; feature 27 is a zero pad.
  3. Pairwise interactions: DVE bf16 2x-mode multiplies (in0 = tile i,
     in1 = tile i+delta covers pairs (i,i+d) AND (i+1,i+1+d) in one op),
     then the per-pair 64-d sums are done on the PE as matmuls with a
     sliding-window "scatter-ones" stationary operand (every window is a
     128-col slice of one 254-col constant), accumulating straight into
     feature-major [pair-row, sample] PSUM tiles.
  4. Top MLP in bf16, feature-major throughout.
"""

import sys
from collections import deque

for _p in ("/opt/trn_rl_repo",):
    if _p not in sys.path:
        sys.path.insert(0, _p)

import numpy as np

import bass_rust
import concourse.bass as bass
import concourse.mybir as mybir
import concourse.tile as tile

# Problem constants (hardcoded per spec nn_DLRM_5403068858958)
B, CD, NF, V, D = 16384, 13, 26, 100000, 64
NCORES = 8
BC = B // NCORES          # 2048 samples per core
BN_INV = 1.0 / np.sqrt(1.0 + 1e-5)
P = 128
NE = NF + 1               # 27 features (bottom + 26 embeddings)
NT = 27                   # overlapped feature-major tiles (t, t+1)
NSLOT = 28                # allg slots: 0 unused/garbage, 1..26 embs, 27 zero
NPAIR = NE * (NE - 1) // 2  # 351 real pairs
F32 = mybir.dt.float32
BF16 = mybir.dt.bfloat16
I32 = mybir.dt.int32
HB = 512                  # samples per interaction/top-MLP chunk
GP_BUFS = 12              # allg ring depth
TR_MODE = "pe"            # 'pe' | 'dma' transpose path
F = mybir.ActivationFunctionType


def _pair_rows():
    """Even-padded, delta-major pair-row layout.

    Returns (evenoff, total_rows, ref_of_row) where pair (i, i+d) sits at
    row evenoff[d] + i, phantom rows hold -1."""
    evenoff = np.zeros(NE, dtype=np.int64)  # evenoff[d], d = 1..26
    off = 0
    for d in range(1, NE):
        evenoff[d] = off
        n = NE - d
        off += ((n + 1) // 2) * 2
    total = off  # 364
    ref = {}
    k = 0
    for i in range(NE):
        for j in range(i + 1, NE):
            ref[(i, j)] = k
            k += 1
    ref_of_row = -np.ones(total, dtype=np.int64)
    for d in range(1, NE):
        for i in range(NE - d):
            ref_of_row[evenoff[d] + i] = ref[(i, i + d)]
    return evenoff, total, ref_of_row


EVENOFF, NROWS, REF_OF_ROW = _pair_rows()   # NROWS = 364
KI = 3                                       # inter k-tiles (384 rows)
KF = 14                                      # flat k-tiles (even fm tiles)
KT = KF + KI                                 # 17 k-tiles into top MLP L1


def _split_multiwaits(nc):
    """The walrus build here accepts at most ONE sync wait per instruction.
    Hoist extra waits onto single-wait NoOps inserted immediately before the
    carrying instruction on the same engine."""
    n_extra = 0
    for fn in nc.m.functions:
        for blk in fn.blocks:
            insts = blk.instructions
            out = []
            for inst in insts:
                si = inst.sync_info
                waits = list(si.on_wait) if si is not None else []
                if len(waits) > 1:
                    for k, w in enumerate(waits[:-1]):
                        nop = bass_rust.InstNoOp(name=f"{inst.name}-sw{k}")
                        nop.engine = inst.engine
                        nop.bass_nofuse = True
                        nop.sync_info = bass_rust.SyncInfo(
                            on_wait=[w], on_update=[])
                        nc.register_instruction(nop, overwrite=True)
                        out.append(nop)
                        n_extra += 1
                    inst.sync_info = bass_rust.SyncInfo(
                        on_wait=[waits[-1]], on_update=list(si.on_update))
                out.append(inst)
            blk.instructions = out
    return n_extra


def _mult_list():
    """(delta, i) product-tile list, i even: covers pairs (i,i+d),(i+1,i+1+d).
    Tiles reading fm tile 0 (i == 0, which needs the bottom MLP) go last so
    the DVE isn't head-blocked on the bottom-MLP chain."""
    out = []
    for d in range(1, NE):
        n = NE - d
        for i in range(2, n, 2):
            out.append((d, i))
    for d in range(1, NE):
        out.append((d, 0))
    return out


MULTS = _mult_list()   # 182 product tiles
FUSE = 2               # product tiles fused per DVE multiply


def _mult_groups():
    """Fused multiply groups: (d, i0, m) covers product tiles
    (d, i0), (d, i0+2), ..., (d, i0+2(m-1)) in one strided tensor_tensor.
    Groups whose i0 == 0 read fm tile 0 (bottom MLP) and go last."""
    head, tail = [], []
    for d in range(1, NE):
        n = NE - d
        evens = list(range(0, n, 2))
        k = 0
        while k < len(evens):
            grp = evens[k:k + FUSE]
            (tail if grp[0] == 0 else head).append((d, grp[0], len(grp)))
            k += FUSE
    return head + tail


MGROUPS = _mult_groups()


def build_nc(b_core=BC, hb=HB, loop_n=1, ablate=(), gp_mult_every=0,
             tr_mode=None):
    """Per-core Bass kernel. gp_mult_every: every k-th interaction multiply
    goes to GPSIMD instead of DVE (0 = none). tr_mode: 'dma' bounces gathered
    tiles through DRAM and uses the xbar transpose; 'pe' transposes on the
    tensor engine straight from SBUF (PSUM -> ACT copy into fm)."""
    if tr_mode is None:
        tr_mode = TR_MODE
    assert b_core % hb == 0 and hb % P == 0
    n_chunk = b_core // hb
    tiles_per_chunk = hb // P
    n_tiles_all = b_core // P

    nc = bass.Bass()
    # ---- DRAM I/O ----
    xT = nc.dram_tensor("xT", [CD, b_core], BF16, kind="ExternalInput")
    # 28 index columns: col 0 -> zero row (slot 0 pad), cols 1-26 real
    # tables, col 27 -> zero row (feature-27 pad)
    idx = nc.dram_tensor("idx", [b_core, NSLOT], I32, kind="ExternalInput")
    tabs = nc.dram_tensor("tabs", [NF * V + 1, D], BF16, kind="ExternalInput")
    w1 = nc.dram_tensor("w1", [CD, 256], BF16, kind="ExternalInput")
    b1 = nc.dram_tensor("b1", [P, 2], F32, kind="ExternalInput")
    w2 = nc.dram_tensor("w2", [P, 2, P], BF16, kind="ExternalInput")
    b2 = nc.dram_tensor("b2", [P, 1], F32, kind="ExternalInput")
    w3 = nc.dram_tensor("w3", [P, D], BF16, kind="ExternalInput")
    b3 = nc.dram_tensor("b3", [D, 1], F32, kind="ExternalInput")
    w4 = nc.dram_tensor("w4", [P, KT, 512], BF16, kind="ExternalInput")
    b4 = nc.dram_tensor("b4", [P, 4], F32, kind="ExternalInput")
    w5 = nc.dram_tensor("w5", [P, 4, 256], BF16, kind="ExternalInput")
    b5 = nc.dram_tensor("b5", [P, 2], F32, kind="ExternalInput")
    w6 = nc.dram_tensor("w6", [P, 2, P], BF16, kind="ExternalInput")
    b6 = nc.dram_tensor("b6", [P, 1], F32, kind="ExternalInput")
    w7 = nc.dram_tensor("w7", [P, 1], BF16, kind="ExternalInput")
    b7 = nc.dram_tensor("b7", [1, 1], F32, kind="ExternalInput")
    sbig = nc.dram_tensor("sbig", [P, 254], BF16, kind="ExternalInput")
    scT = nc.dram_tensor("scT", [1, b_core], F32, kind="ExternalOutput")
    # DRAM staging for gathered tiles: the xbar transpose corrupts data with
    # an SBUF source on HW (verified), so bounce sample-major tiles through
    # DRAM and transpose DRAM -> SBUF (bit-exact on HW).
    ascr = nc.dram_tensor("ascr", [n_tiles_all, P, NSLOT * D], BF16,
                          kind="Internal")

    with tile.TileContext(nc) as tc:
        with (
            tc.tile_pool(name="const", bufs=1) as constp,
            tc.tile_pool(name="fm", bufs=3) as fmp,
            tc.tile_pool(name="g", bufs=GP_BUFS) as gp,
            tc.tile_pool(name="pr", bufs=12) as prp,
            tc.tile_pool(name="acts", bufs=2) as actp,
            tc.tile_pool(name="fmi", bufs=2) as fmip,
            tc.tile_pool(name="outp", bufs=2) as outp,
            tc.tile_pool(name="mmps", bufs=3, space="PSUM") as mmps,
            tc.tile_pool(name="ips", bufs=3, space="PSUM") as ipsp,
            tc.tile_pool(name="tps", bufs=2, space="PSUM") as tps,
        ):
            if tr_mode == "pe":
                from concourse.masks import make_identity
                ident = None
            # ---- gather indices, one early DMA ----
            idxall = constp.tile([P, n_tiles_all, NSLOT], I32)
            nc.sync.dma_start(
                idxall[:], idx[:].rearrange("(t p) f -> p t f", p=P))

            # ---- persistent constants / weights ----
            sbigs = constp.tile([P, 254], BF16)
            nc.sync.dma_start(sbigs[:], sbig[:])
            if tr_mode == "pe":
                from concourse.masks import make_identity
                identt = constp.tile([P, P], BF16)
                make_identity(nc, identt[:])
            w1s = constp.tile([CD, 256], BF16)
            nc.sync.dma_start(w1s[:], w1[:])
            b1s = constp.tile([P, 2], F32)
            nc.sync.dma_start(b1s[:], b1[:])
            w2s = constp.tile([P, 2, P], BF16)
            nc.sync.dma_start(w2s[:], w2[:])
            b2s = constp.tile([P, 1], F32)
            nc.sync.dma_start(b2s[:], b2[:])
            w3s = constp.tile([P, D], BF16)
            nc.sync.dma_start(w3s[:], w3[:])
            b3s = constp.tile([D, 1], F32)
            nc.sync.dma_start(b3s[:], b3[:])
            w4s = constp.tile([P, KT, 512], BF16)
            nc.sync.dma_start(w4s[:], w4[:])
            b4s = constp.tile([P, 4], F32)
            nc.sync.dma_start(b4s[:], b4[:])
            w5s = constp.tile([P, 4, 256], BF16)
            nc.sync.dma_start(w5s[:], w5[:])
            b5s = constp.tile([P, 2], F32)
            nc.sync.dma_start(b5s[:], b5[:])
            w6s = constp.tile([P, 2, P], BF16)
            nc.sync.dma_start(w6s[:], w6[:])
            b6s = constp.tile([P, 1], F32)
            nc.sync.dma_start(b6s[:], b6[:])
            w7s = constp.tile([P, 1], BF16)
            nc.sync.dma_start(w7s[:], w7[:])
            b7s = constp.tile([1, 1], F32)
            nc.sync.dma_start(b7s[:], b7[:])

            def emit_body(iv=None):
                xTs = actp.tile([CD, b_core], BF16, tag="xTs", bufs=1)
                nc.sync.dma_start(xTs[:], xT[:])

                # ---------- bottom MLP, feature-major, whole core ----------
                bT = actp.tile([D, b_core], BF16, tag="bT", bufs=1)
                BW = 512
                for nck in range(b_core // BW):
                    nsl = slice(nck * BW, (nck + 1) * BW)
                    h1 = actp.tile([P, 2, BW], BF16, tag="h1")
                    h2 = actp.tile([P, BW], BF16, tag="h2")
                    for mc in range(2):
                        ps = mmps.tile([P, BW], F32, tag="mmps")
                        nc.tensor.matmul(
                            ps[:], w1s[:, mc * P:(mc + 1) * P], xTs[:, nsl],
                            start=True, stop=True)
                        nc.scalar.activation(
                            h1[:, mc, :], ps[:], F.Relu, bias=b1s[:, mc:mc + 1])
                    ps = mmps.tile([P, BW], F32, tag="mmps")
                    for kc in range(2):
                        nc.tensor.matmul(
                            ps[:], w2s[:, kc, :], h1[:, kc, :],
                            start=(kc == 0), stop=(kc == 1))
                    nc.scalar.activation(
                        h2[:], ps[:], F.Relu, bias=b2s[:, 0:1])
                    ps = mmps.tile([P, BW], F32, tag="mmps")
                    nc.tensor.matmul(
                        ps[:D], w3s[:], h2[:], start=True, stop=True)
                    nc.scalar.activation(
                        bT[:, nsl], ps[:D], F.Identity, bias=b3s[:])

                # ---------- per chunk: interactions + top MLP ----------
                # fm buffers are PER CHUNK so transposes for chunk h+1 don't
                # WAR-serialize against reads of chunk h. The top MLP of
                # chunk h is emitted as fine-grained thunks INTERLEAVED with
                # chunk h+1's scatter MMs: each scatter MM is gated by its
                # DVE multiply (327 ns vs 213 ns MM), so PE has slack for
                # one MLP matmul every ~2 scatter MMs. Without this the two
                # engines convoy (PE drains the product ring then sleeps).
                def _mlp_thunks(h, xs, fm, fmi):
                    th = []
                    box = {}

                    def _alloc(name, shape, dt=BF16):
                        def f():
                            box[name] = actp.tile(shape, dt, tag=name,
                                                  name=f"{name}_{h}")
                        return f

                    def _ps(name):
                        def f():
                            box[name] = mmps.tile([P, hb], F32, tag="mmps",
                                                  name=f"{name}_{h}")
                        return f

                    th.append(_alloc("t1", [P, 4, hb]))
                    th.append(_alloc("t2", [P, 2, hb]))
                    th.append(_alloc("t3", [P, hb]))
                    for mc in range(4):
                        th.append(_ps(f"ps1_{mc}"))
                        for kc in range(KF):
                            def f(mc=mc, kc=kc):
                                nc.tensor.matmul(
                                    box[f"ps1_{mc}"][:],
                                    w4s[:, kc, mc * P:(mc + 1) * P],
                                    fm[:, 2 * kc, :],
                                    start=(kc == 0), stop=False)
                            th.append(f)
                        for kc in range(KI):
                            def f(mc=mc, kc=kc):
                                nc.tensor.matmul(
                                    box[f"ps1_{mc}"][:],
                                    w4s[:, KF + kc, mc * P:(mc + 1) * P],
                                    fmi[:, kc, :],
                                    start=False, stop=(kc == KI - 1))
                            th.append(f)

                        def f(mc=mc):
                            nc.scalar.activation(
                                box["t1"][:, mc, :], box[f"ps1_{mc}"][:],
                                F.Relu, bias=b4s[:, mc:mc + 1])
                        th.append(f)
                    for mc in range(2):
                        th.append(_ps(f"ps2_{mc}"))
                        for kc in range(4):
                            def f(mc=mc, kc=kc):
                                nc.tensor.matmul(
                                    box[f"ps2_{mc}"][:],
                                    w5s[:, kc, mc * P:(mc + 1) * P],
                                    box["t1"][:, kc, :],
                                    start=(kc == 0), stop=(kc == 3))
                            th.append(f)

                        def f(mc=mc):
                            nc.scalar.activation(
                                box["t2"][:, mc, :], box[f"ps2_{mc}"][:],
                                F.Relu, bias=b5s[:, mc:mc + 1])
                        th.append(f)
                    th.append(_ps("ps3"))
                    for kc in range(2):
                        def f(kc=kc):
                            nc.tensor.matmul(
                                box["ps3"][:], w6s[:, kc, :],
                                box["t2"][:, kc, :],
                                start=(kc == 0), stop=(kc == 1))
                        th.append(f)

                    def f():
                        nc.scalar.activation(
                            box["t3"][:], box["ps3"][:], F.Relu,
                            bias=b6s[:, 0:1])
                    th.append(f)
                    th.append(_ps("ps7"))

                    def f():
                        nc.tensor.matmul(
                            box["ps7"][:1], w7s[:], box["t3"][:],
                            start=True, stop=True)
                    th.append(f)

                    def f():
                        so = outp.tile([1, hb], F32, tag="so",
                                       name=f"so_{h}")
                        nc.scalar.activation(
                            so[:], box["ps7"][:1], F.Identity, bias=b7s[:])
                        nc.sync.dma_start(scT[:, xs], so[:])
                    th.append(f)
                    return th

                pending_mlp = deque()
                for h in range(n_chunk):
                    xs = slice(h * hb, (h + 1) * hb)

                    # -- gather + transpose for this chunk's sample tiles --
                    fm = fmp.tile([P, NT, hb], BF16, tag="fmc")
                    if 'gather' not in ablate:
                        for tl in range(tiles_per_chunk):
                            t = h * tiles_per_chunk + tl
                            col = slice(tl * P, (tl + 1) * P)
                            allg = gp.tile([P, NSLOT, D], BF16, tag="allg")
                            # zero-pad slots: 0 only feeds fm0's lower half
                            # (overwritten by the bottom-MLP copy); 27 is the
                            # feature-27 zero pad
                            nc.vector.memset(allg[:, 0, :], 0.0)
                            nc.vector.memset(allg[:, NE, :], 0.0)
                            # HW only honors one index per partition per
                            # indirect DMA (multi-column offsets verified
                            # broken) -> one gather per real slot
                            for sl in range(1, NF + 1):
                                nc.gpsimd.indirect_dma_start(
                                    out=allg[:, sl, :], out_offset=None,
                                    in_=tabs[:],
                                    in_offset=bass.IndirectOffsetOnAxis(
                                        ap=idxall[:, t, sl:sl + 1], axis=0))
                            if tr_mode == "dma":
                                # store on SP with the transposes: SP's FIFO
                                # order matches data-arrival order (ACT's
                                # stream would head-block it)
                                nc.sync.dma_start(
                                    ascr[t],
                                    allg[:].rearrange("p a b -> p (a b)"))
                            if 'tr' in ablate:
                                continue
                            if tr_mode == "dma":
                                # one DmaTransposeAnt with a 3D out does k
                                # independent 128x128 block transposes
                                nc.sync.dma_start(
                                    fm[:, 0:NT:2, col],
                                    ascr[t, :, 0:14 * P],
                                    transpose=True)
                                nc.sync.dma_start(
                                    fm[:, 1:NT:2, col],
                                    ascr[t, :, D:D + 13 * P],
                                    transpose=True)
                            else:
                                for f in range(NT):
                                    tp = tps.tile([P, P], BF16, tag="tp",
                                                  name=f"tp{t}_{f}")
                                    nc.tensor.transpose(
                                        tp[:], allg[:, f:f + 2, :], identt[:])
                                    nc.scalar.activation(
                                        fm[:, f, col], tp[:], F.Copy)
                    # bottom-MLP rows -> fm tile 0 lower half (after the
                    # block-0 transposes, which write zeros/garbage there)
                    nc.scalar.activation(
                        fm[0:D, 0, :], bT[:, xs], F.Copy)

                    # -- interaction multiplies + scatter-ones PE reduce --
                    fmi = fmip.tile([P, KI, hb], BF16, tag="fmi")
                    ips = []
                    for _ik in range(KI):
                        ipst = ipsp.tile([P, hb], F32, tag="ips", name=f"ips{h}_{_ik}")
                        ips.append(ipst)
                    if 'inter' not in ablate:
                        # group product tiles by psum k-tile for start/stop
                        by_kt = [[] for _ in range(KI)]
                        for (d, i) in MULTS:
                            r = int(EVENOFF[d]) + i
                            by_kt[r // P].append((d, i, r % P))
                        # start/stop are per (kt, 32-row band): col-tiled
                        # M=32 matmuls cut LDWEIGHTS to 32 columns and let
                        # the PE overlap band tiles (col-group concurrency)
                        n_mult = 0
                        for kt in range(KI):
                            n_in_kt = len(by_kt[kt])
                            for j, (d, i, lr) in enumerate(by_kt[kt]):
                                pr = prp.tile([P, hb], BF16, tag="pr")
                                n_mult += 1
                                eng = (nc.gpsimd if gp_mult_every and
                                       (n_mult % gp_mult_every == 0)
                                       else nc.vector)
                                eng.tensor_tensor(
                                    pr[:], fm[:, i, :], fm[:, i + d, :],
                                    op=mybir.AluOpType.mult)
                                nc.tensor.matmul(
                                    ips[kt][:], sbigs[:, 126 - lr:254 - lr],
                                    pr[:],
                                    start=(j == 0), stop=(j == n_in_kt - 1))
                                # slot one prior-chunk MLP thunk into every
                                # other scatter gap
                                if n_mult % 2 == 0 and pending_mlp:
                                    pending_mlp.popleft()()
                            nc.scalar.activation(
                                fmi[:, kt, :], ips[kt][:], F.Copy)
                    else:
                        while pending_mlp:
                            pending_mlp.popleft()()
                        for kt in range(KI):
                            nc.vector.memset(fmi[:, kt, :], 0.0)

                    while pending_mlp:
                        pending_mlp.popleft()()
                    pending_mlp.extend(_mlp_thunks(h, xs, fm, fmi))

                while pending_mlp:
                    pending_mlp.popleft()()

            for _rep in range(loop_n):
                emit_body()

    _split_multiwaits(nc)
    return nc


def prep_host(inputs, b_core=BC):
    """Fold BN, cast to bf16, build per-core input maps."""
    import ml_dtypes
    bf16 = ml_dtypes.bfloat16

    f = lambda a: np.ascontiguousarray(np.asarray(a), dtype=np.float32)
    fb = lambda a: np.ascontiguousarray(np.asarray(a, dtype=np.float32)
                                        .astype(bf16))
    continuous = f(inputs["continuous"])
    cat_idx = np.asarray(inputs["cat_idx"])
    tabs = np.asarray(inputs["emb_tables"], dtype=np.float32).reshape(NF * V, D)
    tabs = fb(np.concatenate([tabs, np.zeros((1, D), np.float32)], axis=0))

    g = lambda a: np.ascontiguousarray(a, dtype=np.float32)
    s1 = f(inputs["g1"]) * np.float32(BN_INV)
    w1f = fb(f(inputs["W1"]) * s1[None, :])
    b1f = g((f(inputs["b1"]) * s1 + f(inputs["be1"])).reshape(2, P).T)
    s2 = f(inputs["g2"]) * np.float32(BN_INV)
    w2f = fb((f(inputs["W2"]) * s2[None, :]).reshape(2, P, P)
             .transpose(1, 0, 2).copy())
    b2f = g((f(inputs["b2"]) * s2 + f(inputs["be2"])).reshape(1, P).T)
    w3f = fb(inputs["W3"])
    b3f = f(inputs["b3"]).reshape(D, 1)

    # W4: rows [0:351] inter (ref order), [351:2079] flat feature f at
    # 351 + f*64. Rearrange to my k-tile order.
    s4 = f(inputs["g4"]) * np.float32(BN_INV)
    W4 = f(inputs["W4"]) * s4[None, :]
    W4m = np.zeros((KT * P, 512), dtype=np.float32)
    W4m[:NE * D] = W4[NPAIR:NPAIR + NE * D]      # flat: features 0..26
    for r in range(NROWS):                        # inter rows, my layout
        ref = REF_OF_ROW[r]
        if ref >= 0:
            W4m[KF * P + r] = W4[ref]
    w4f = fb(W4m.reshape(KT, P, 512).transpose(1, 0, 2).copy())
    b4f = g((f(inputs["b4"]) * s4 + f(inputs["be4"])).reshape(4, P).T)
    s5 = f(inputs["g5"]) * np.float32(BN_INV)
    w5f = fb((f(inputs["W5"]) * s5[None, :]).reshape(4, P, 256)
             .transpose(1, 0, 2).copy())
    b5f = g((f(inputs["b5"]) * s5 + f(inputs["be5"])).reshape(2, P).T)
    s6 = f(inputs["g6"]) * np.float32(BN_INV)
    w6f = fb((f(inputs["W6"]) * s6[None, :]).reshape(2, P, P)
             .transpose(1, 0, 2).copy())
    b6f = g((f(inputs["b6"]) * s6 + f(inputs["be6"])).reshape(1, P).T)
    w7f = fb(inputs["W7"])
    b7f = f(inputs["b7"]).reshape(1, 1)

    sbig_np = np.zeros((P, 254), dtype=np.float32)
    sbig_np[0:D, 126] = 1.0
    sbig_np[D:P, 127] = 1.0
    sbig_np = sbig_np.astype(bf16)

    foffs = (np.arange(NF, dtype=np.int64) * V).astype(np.int32)
    in_maps = []
    ncores = B // b_core
    for c in range(ncores):
        sl = slice(c * b_core, (c + 1) * b_core)
        in_maps.append(dict(
            xT=np.ascontiguousarray(continuous[sl].T).astype(bf16),
            idx=np.ascontiguousarray(np.concatenate(
                [np.full((b_core, 1), NF * V, np.int32),
                 cat_idx[sl].astype(np.int32) + foffs[None, :],
                 np.full((b_core, 1), NF * V, np.int32)], axis=1)),
            tabs=tabs,
            w1=w1f, b1=b1f, w2=w2f, b2=b2f, w3=w3f, b3=b3f,
            w4=w4f, b4=b4f, w5=w5f, b5=b5f, w6=w6f, b6=b6f,
            w7=w7f, b7=b7f, sbig=sbig_np,
        ))
    return in_maps


_NC_CACHE = {}


def kernel(**inputs) -> np.ndarray:
    from concourse.bass_utils import run_bass_kernel_spmd

    key = (BC, HB)
    if key not in _NC_CACHE:
        _NC_CACHE[key] = build_nc(*key)
    nc = _NC_CACHE[key]
    in_maps = prep_host(inputs, BC)
    res = run_bass_kernel_spmd(nc, in_maps, core_ids=list(range(NCORES)))
    out = np.concatenate(
        [r["scT"].reshape(BC, 1) for r in res.results], axis=0)
    return out.astype(np.float32)

